# revision 6
# baseline (speedup 1.0000x reference)
"""Trainium2 Bass kernel for PhysicsInformedMHDSolver.

Data-parallel over 8 NeuronCores: each core runs batch shard of 2048 rows
through MLP (8->256->512->256->4096, gelu+LN via erf + folded affine),
tanh -> segmented cummax (single DVE scan w/ additive reset mask) ->
sigmoid (+row-sum accum) -> 5-pt Laplacian residual accum.
Host combines per-row partials into pressure / physics_loss and builds the
constant q_profile.
"""

import numpy as np

GRID = 64
NG = GRID * GRID          # 4096
B = 16384
NCORES = 8
BS = B // NCORES          # 2048 per core
P = 128
NT = BS // P              # 16 tiles per core
SQ2INV = 0.7071067811865476
EPS = 1e-5
HID = [256, 512, 256]

_CACHE = {}

# results of the last device run (test.py reads this for profiling)
LAST_RESULTS = None


def _build_bass():
    import concourse.bacc as bacc
    import concourse.mybir as mybir
    import concourse.tile as tile
    from contextlib import ExitStack

    dt = mybir.dt
    f32 = dt.float32
    u32 = dt.uint32
    bf16 = dt.bfloat16
    Alu = mybir.AluOpType
    Act = mybir.ActivationFunctionType

    nc = bacc.Bacc()

    # ---- DRAM I/O ----
    pT = nc.dram_tensor("pT", [9, BS], f32, kind="ExternalInput")       # plasma^T + ones row
    negs = nc.dram_tensor("negs", [BS, 1], f32, kind="ExternalInput")   # -plasma[:,0]
    we = nc.dram_tensor("we", [9, HID[0]], f32, kind="ExternalInput")   # [W_enc; b_enc]
    w1 = nc.dram_tensor("w1", [HID[0], HID[1]], f32, kind="ExternalInput")
    a1w = nc.dram_tensor("a1w", [2, HID[1]], f32, kind="ExternalInput")  # [colsum; bias']
    w2 = nc.dram_tensor("w2", [HID[1], HID[2]], f32, kind="ExternalInput")
    a2w = nc.dram_tensor("a2w", [2, HID[2]], f32, kind="ExternalInput")
    wf = nc.dram_tensor("wf", [HID[2], NG], f32, kind="ExternalInput")
    afw = nc.dram_tensor("afw", [2, NG], f32, kind="ExternalInput")
    ident = nc.dram_tensor("ident", [P, P], f32, kind="ExternalInput")
    mask = nc.dram_tensor("mask", [P, NG], f32, kind="ExternalInput")   # scan reset mask
    oflux = nc.dram_tensor("oflux", [BS, NG], f32, kind="ExternalOutput")
    ofsum = nc.dram_tensor("ofsum", [P, NT], f32, kind="ExternalOutput")
    ogs = nc.dram_tensor("ogs", [P, NT], f32, kind="ExternalOutput")

    with ExitStack() as ctx:
        tc = ctx.enter_context(tile.TileContext(nc))
        const = ctx.enter_context(tc.tile_pool(name="const", bufs=1))
        pst = ctx.enter_context(tc.tile_pool(name="pst", bufs=3))
        pwork = ctx.enter_context(tc.tile_pool(name="pwork", bufs=2))
        pbig = ctx.enter_context(tc.tile_pool(name="pbig", bufs=2))
        pfl = ctx.enter_context(tc.tile_pool(name="pfl", bufs=2))
        plap = ctx.enter_context(tc.tile_pool(name="plap", bufs=2))
        ppz = ctx.enter_context(tc.tile_pool(name="ppz", bufs=2, space="PSUM"))
        ppt = ctx.enter_context(tc.tile_pool(name="ppt", bufs=2, space="PSUM"))
        ppf = ctx.enter_context(tc.tile_pool(name="ppf", bufs=2, space="PSUM"))

        # ---- load constants into SBUF ----
        we_sb = const.tile([9, HID[0]], f32)
        nc.sync.dma_start(we_sb[:], we[:])
        w1_sb = []
        for j in range(2):
            w1j = const.tile([P, HID[1]], f32, tag=f"w1_{j}", name=f"w1_{j}")
            nc.sync.dma_start(w1j[:], w1[j * P:(j + 1) * P, :])
            w1_sb.append(w1j)
        a1w_sb = const.tile([2, HID[1]], f32)
        nc.sync.dma_start(a1w_sb[:], a1w[:])
        w2_sb = []
        for j in range(4):
            w2j = const.tile([P, HID[2]], f32, tag=f"w2_{j}", name=f"w2_{j}")
            nc.sync.dma_start(w2j[:], w2[j * P:(j + 1) * P, :])
            w2_sb.append(w2j)
        a2w_sb = const.tile([2, HID[2]], f32)
        nc.sync.dma_start(a2w_sb[:], a2w[:])
        wf_sb = []
        for j in range(2):
            wfj = const.tile([P, NG], f32, tag=f"wf_{j}", name=f"wf_{j}")
            nc.sync.dma_start(wfj[:], wf[j * P:(j + 1) * P, :])
            wf_sb.append(wfj)
        afw_sb = const.tile([2, NG], f32)
        nc.sync.dma_start(afw_sb[:], afw[:])
        ident_sb = const.tile([P, P], f32)
        nc.sync.dma_start(ident_sb[:], ident[:])
        mask_sb = const.tile([P, NG], f32)
        nc.sync.dma_start(mask_sb[:], mask[:])

        one_i = const.tile([P, 1], u32)
        nc.vector.memset(one_i[:], 1)
        magic_i = const.tile([P, 1], u32)
        nc.vector.memset(magic_i[:], 0x5F3759DF)

        fsum_acc = const.tile([P, NT], f32)
        gs_acc = const.tile([P, NT], f32)

        # PE warm-reads of every weight tile: makes the tensor engine's
        # vector clock observe all const-DMA queue semaphores once, so the
        # real matmul groups don't exceed the per-instruction sync-wait cap.
        pe_read = [we_sb, *w1_sb, a1w_sb, *w2_sb, a2w_sb, *wf_sb, afw_sb,
                   ident_sb]
        for i, cts in enumerate(pe_read):
            dz = ppt.tile([1, 1], f32, tag="tp", name=f"dz{i}")
            nc.tensor.matmul(dz[:], lhsT=cts[0:1, 0:1], rhs=cts[0:1, 0:1],
                             start=True, stop=True)

        def gelu_stats(z, n, lname, erf_scale, ve_prev=None):
            """z: PSUM [P,n] pre-activation (up to per-row scale).
            Returns (u, aug_s, rs, ve): u = (1+erf(z*s))*z (scaled gelu),
            aug_s [P,2] = (-mean, S), rs [P,1] = R.  The eps term is scaled
            by the carried per-row factor: ve = var + 4*eps*ve_prev."""
            e = pwork.tile([P, n], f32, tag=f"e{lname}", name=f"e{lname}")
            nc.scalar.activation(e[:], z[:], Act.Erf, bias=0.0, scale=erf_scale)
            u = pwork.tile([P, n], f32, tag=f"u{lname}", name=f"u{lname}")
            nc.vector.scalar_tensor_tensor(u[:], e[:], 1.0, z[:], Alu.add, Alu.mult)
            st = pst.tile([P, 6], f32, tag="st", name="st")
            nc.vector.bn_stats(st[:], u[:])
            mv = pst.tile([P, 2], f32, tag="mv", name="mv")
            nc.vector.bn_aggr(mv[:], st[:])
            ve = pst.tile([P, 1], f32, tag=f"ve{lname}", name=f"ve{lname}")
            if ve_prev is None:
                nc.vector.tensor_scalar_add(ve[:], mv[:, 1:2], 4.0 * EPS)
            else:
                nc.vector.scalar_tensor_tensor(ve[:], ve_prev[:], 4.0 * EPS,
                                               mv[:, 1:2], Alu.mult, Alu.add)
            # rsqrt via magic-constant + 3 Newton steps (all DVE, no ACT table swap)
            ish = pst.tile([P, 1], u32, tag="ish", name="ish")
            nc.vector.tensor_tensor(ish[:], ve[:].bitcast(u32), one_i[:],
                                    Alu.logical_shift_right)
            y = pst.tile([P, 1], f32, tag="y", name="y")
            nc.vector.tensor_tensor(y[:].bitcast(u32), magic_i[:], ish[:], Alu.subtract)
            tn = pst.tile([P, 1], f32, tag="tn", name="tn")
            for _ in range(3):
                nc.vector.tensor_mul(tn[:], y[:], y[:])
                nc.vector.tensor_mul(tn[:], tn[:], ve[:])
                nc.vector.tensor_scalar(tn[:], tn[:], -0.5, 1.5, Alu.mult, Alu.add)
                nc.vector.tensor_mul(y[:], y[:], tn[:])
            aug_s = pst.tile([P, 2], f32, tag="aug_s", name="aug_s")
            nc.vector.tensor_scalar_mul(aug_s[:, 0:1], mv[:, 0:1], -1.0)
            nc.vector.tensor_mul(aug_s[:, 1:2], ve[:], y[:])   # S = ve*R
            return u, aug_s, y, ve

        def transpose_set(u, nblk, aug_s, lname):
            """Transpose u [P, nblk*P] -> uT blocks; aug_s [P,2] -> augT [2,P]."""
            uT = pwork.tile([P, nblk * P], f32, tag=f"uT{lname}", name=f"uT{lname}")
            for j in range(nblk):
                tp = ppt.tile([P, P], f32, tag="tp", name="tp")
                nc.tensor.transpose(tp[:], u[:, j * P:(j + 1) * P], ident_sb[:])
                nc.scalar.copy(uT[:, j * P:(j + 1) * P], tp[:])
            tpa = ppt.tile([2, P], f32, tag="tp", name="tpa")
            nc.tensor.transpose(tpa[:], aug_s[:], ident_sb[:])
            augT = pwork.tile([2, P], f32, tag="augT", name="augT")
            nc.scalar.copy(augT[:], tpa[:])
            return uT, augT

        for t in range(NT):
            # ---- encoder ----
            xT = pwork.tile([9, P], f32, tag="xT", name="xT")
            nc.sync.dma_start(xT[:], pT[:, t * P:(t + 1) * P])
            z1 = ppz.tile([P, HID[0]], f32, tag="z", name="z1")
            nc.tensor.matmul(z1[:], lhsT=xT[:], rhs=we_sb[:], start=True, stop=True)
            u1, aug1, rs1, ve1 = gelu_stats(z1, HID[0], "1", SQ2INV)
            rssc1 = pst.tile([P, 1], f32, tag="rssc", name="rssc1")
            nc.vector.tensor_scalar_mul(rssc1[:], rs1[:], SQ2INV)

            # ---- layer 1: 256 -> 512 ----
            u1T, aug1T = transpose_set(u1, 2, aug1, "1")
            z2 = ppz.tile([P, HID[1]], f32, tag="z", name="z2")
            for j in range(2):
                nc.tensor.matmul(z2[:], lhsT=u1T[:, j * P:(j + 1) * P],
                                 rhs=w1_sb[j][:], start=(j == 0), stop=False)
            nc.tensor.matmul(z2[:], lhsT=aug1T[:], rhs=a1w_sb[:], start=False, stop=True)
            u2, aug2, rs2, ve2 = gelu_stats(z2, HID[1], "2", rssc1[:], ve1)
            rssc2 = pst.tile([P, 1], f32, tag="rssc", name="rssc2")
            nc.vector.tensor_scalar_mul(rssc2[:], rs2[:], SQ2INV)

            # ---- layer 2: 512 -> 256 ----
            u2T, aug2T = transpose_set(u2, 4, aug2, "2")
            z3 = ppz.tile([P, HID[2]], f32, tag="z", name="z3")
            for j in range(4):
                nc.tensor.matmul(z3[:], lhsT=u2T[:, j * P:(j + 1) * P],
                                 rhs=w2_sb[j][:], start=(j == 0), stop=False)
            nc.tensor.matmul(z3[:], lhsT=aug2T[:], rhs=a2w_sb[:], start=False, stop=True)
            u3, aug3, rs3, ve3 = gelu_stats(z3, HID[2], "3", rssc2[:], ve2)

            # ---- final layer: 256 -> 4096, tanh(scale=rs3) ----
            u3T, aug3T = transpose_set(u3, 2, aug3, "3")
            fy = pbig.tile([P, NG], f32, tag="fbig", name="fy")
            for q in range(4):
                pf = ppf.tile([P, 1024], f32, tag="pf", name="pf")
                for j in range(2):
                    for s in range(2):
                        off = q * 1024 + s * 512
                        nc.tensor.matmul(pf[:, s * 512:(s + 1) * 512],
                                         lhsT=u3T[:, j * P:(j + 1) * P],
                                         rhs=wf_sb[j][:, off:off + 512],
                                         start=(j == 0), stop=False)
                for s in range(2):
                    off = q * 1024 + s * 512
                    nc.tensor.matmul(pf[:, s * 512:(s + 1) * 512], lhsT=aug3T[:],
                                     rhs=afw_sb[:, off:off + 512],
                                     start=False, stop=True)
                nc.scalar.activation(fy[:, q * 1024:(q + 1) * 1024], pf[:],
                                     Act.Tanh, bias=0.0, scale=rs3[:])

            # ---- cummax along each 64-wide row: one segmented scan ----
            fx = pbig.tile([P, NG], f32, tag="fbig", name="fx")
            nc.vector.tensor_tensor_scan(fx[:], mask_sb[:], fy[:], 0.0,
                                         Alu.add, Alu.max)

            # ---- sigmoid (+ row sums) ----
            fl = pfl.tile([P, NG], f32, tag="fl", name="fl")
            nc.scalar.activation(fl[:], fx[:], Act.Sigmoid, bias=0.0, scale=1.0,
                                 accum_out=fsum_acc[:, t:t + 1])
            nc.sync.dma_start(oflux[t * P:(t + 1) * P, :], fl[:])

            # ---- 5-point Laplacian residual on interior ----
            f3 = fl[:].rearrange("p (r c) -> p r c", r=GRID, c=GRID)
            a1t = plap.tile([P, 62, 62], bf16, tag="lapA", name="a1t")
            nc.vector.tensor_add(a1t[:], f3[:, 0:62, 1:63], f3[:, 2:64, 1:63])
            a2t = plap.tile([P, 62, 62], bf16, tag="lapB", name="a2t")
            nc.vector.tensor_add(a2t[:], f3[:, 1:63, 0:62], f3[:, 1:63, 2:64])
            # d = (f * -4) + a1 + a2   (write back into a1t)
            nc.vector.scalar_tensor_tensor(a1t[:], f3[:, 1:63, 1:63], -4.0, a1t[:],
                                           Alu.mult, Alu.add)
            nc.vector.tensor_add(a1t[:], a1t[:], a2t[:])
            ns = pst.tile([P, 1], f32, tag="ns", name="ns")
            nc.sync.dma_start(ns[:], negs[t * P:(t + 1) * P, :])
            # sum over interior of (d - s)^2  (square output discarded into a2t)
            nc.scalar.activation(a2t[:], a1t[:], Act.Square, bias=ns[:], scale=1.0,
                                 accum_out=gs_acc[:, t:t + 1])

        nc.sync.dma_start(ofsum[:], fsum_acc[:])
        nc.sync.dma_start(ogs[:], gs_acc[:])

    if not nc.is_finalized():
        nc.finalize()
    return nc


def _prep_host(inputs):
    f = np.float32
    plasma = np.asarray(inputs["plasma_state"], f)
    W_enc = np.asarray(inputs["W_enc"], f); b_enc = np.asarray(inputs["b_enc"], f)
    g_enc = np.asarray(inputs["g_enc"], f); be_enc = np.asarray(inputs["be_enc"], f)
    W1 = np.asarray(inputs["W1"], f); b1 = np.asarray(inputs["b1"], f)
    g1 = np.asarray(inputs["g1"], f); be1 = np.asarray(inputs["be1"], f)
    W2 = np.asarray(inputs["W2"], f); b2 = np.asarray(inputs["b2"], f)
    g2 = np.asarray(inputs["g2"], f); be2 = np.asarray(inputs["be2"], f)
    Wf = np.asarray(inputs["Wf"], f); bf = np.asarray(inputs["bf"], f)

    we = np.ascontiguousarray(np.vstack([W_enc, b_enc[None, :]]), f)
    w1g = np.ascontiguousarray(g_enc[:, None] * W1, f)
    a1w = np.ascontiguousarray(np.vstack([w1g.sum(0), be_enc @ W1 + b1]), f)
    w2g = np.ascontiguousarray(g1[:, None] * W2, f)
    a2w = np.ascontiguousarray(np.vstack([w2g.sum(0), be1 @ W2 + b2]), f)
    wfg = np.ascontiguousarray(g2[:, None] * Wf, f)
    afw = np.ascontiguousarray(np.vstack([wfg.sum(0), be2 @ Wf + bf]), f)
    ident = np.eye(P, dtype=f)
    mask = np.zeros((P, NG), f)
    mask[:, ::GRID] = -1e30

    shared = dict(we=we, w1=w1g, a1w=a1w, w2=w2g, a2w=a2w, wf=wfg, afw=afw,
                  ident=ident, mask=mask)
    in_maps = []
    for c in range(NCORES):
        sh = plasma[c * BS:(c + 1) * BS]
        pTn = np.ascontiguousarray(
            np.vstack([sh.T, np.ones((1, BS), f)]), f)
        negs = np.ascontiguousarray(-sh[:, 0:1], f)
        m = dict(shared)
        m["pT"] = pTn
        m["negs"] = negs
        in_maps.append(m)
    return in_maps, plasma


def kernel(**inputs):
    global LAST_RESULTS
    from concourse.bass_utils import run_bass_kernel_spmd

    if "nc" not in _CACHE:
        _CACHE["nc"] = _build_bass()
    nc = _CACHE["nc"]

    in_maps, plasma = _prep_host(inputs)
    res = run_bass_kernel_spmd(nc, in_maps, core_ids=list(range(NCORES)))
    LAST_RESULTS = res

    f = np.float32
    flux = np.empty((B, GRID, GRID), f)
    fsum = np.empty(B, f)
    gs_rows = np.empty(B, f)
    for c in range(NCORES):
        out = res.results[c]
        flux[c * BS:(c + 1) * BS] = out["oflux"].reshape(BS, GRID, GRID)
        fsum[c * BS:(c + 1) * BS] = np.ascontiguousarray(out["ofsum"].T).reshape(BS)
        gs_rows[c * BS:(c + 1) * BS] = np.ascontiguousarray(out["ogs"].T).reshape(BS)

    s = plasma[:, 0].astype(np.float64)
    n_bound = NG - 62 * 62    # boundary cells: lap = 0 -> (0 - s)^2 each
    gs_residual = (gs_rows.astype(np.float64).sum()
                   + n_bound * np.square(s).sum()) / (B * NG)
    current_consistency = np.mean(np.square(s - fsum.astype(np.float64)))
    pressure = np.exp(f(-2.0) * fsum / f(NG)).astype(f)
    q = np.linspace(0.8, 3.5, GRID).astype(f)
    q_profile = np.broadcast_to(q, (B, GRID)).copy()
    pw = np.asarray(inputs["physics_weights"], f)
    stability = f(np.mean(np.maximum(f(1.1) - q_profile.min(axis=1), f(0.0))))
    physics_loss = f(pw[0] * f(gs_residual) + pw[1] * f(current_consistency)
                     + pw[2] * stability)
    return flux, q_profile, pressure, physics_loss
